# revision 3
# baseline (speedup 1.0000x reference)
"""Trainium2 Bass kernel for nn_Arch7V3GraphEncoder (gnn_message_passing), v4.

v2 + merged gather/bond matmuls: tiles hold 120 nodes (15 subgraphs), and
partitions 120-127 of every h_nm block carry the bond embedding table, so a
single one-hot stationary [128, slots] per chunk computes
h[src]*valid + bond_emb[tok] in one matmul (half the PE instructions of the
separate gather+bond pair). Subgraph pooling columns are globally ordered, so
the 15-subgraph tiling leaves the HT-softmax reduce untouched.
"""

import sys

sys.path.insert(0, "/opt/trn_rl_repo")

import numpy as np
import ml_dtypes

BF16 = ml_dtypes.bfloat16
F8 = ml_dtypes.float8_e4m3

# Problem constants (hardcoded per spec).
N_TOTAL = 4096
M_SUB = 4
K_NODES = 8
L_LAYERS = 4
H = 128
NUM_GRAPHS = 32
IN_CH = 119
EDGE_DIM = 8
S_ALL = N_TOTAL * M_SUB          # 16384 subgraphs
SK_ALL = S_ALL * K_NODES         # 131072 flat nodes
E_ALL = 12 * S_ALL               # 196608 edges
NCORES = 8
S_LOC = S_ALL // NCORES          # 2048 subgraphs / core
SK_LOC = SK_ALL // NCORES        # 16384 flat nodes / core
NCAN_LOC = N_TOTAL // NCORES     # 512 canonical nodes / core
NQ = NCAN_LOC // 128             # 4 canonical chunks of 128
E_CAP = 256                      # edge slots per tile (2 chunks of 128)

SG_T = 15                        # subgraphs per tile
TN = SG_T * K_NODES              # 120 nodes per tile
NT = (S_LOC + SG_T - 1) // SG_T  # 137 tiles (last tile has 8 subgraphs)
D_W = 2 * TN                     # d_oh columns per tile (2 chunks x 120)

# node-column base and node count of tile t
def _tile_base(t):
    return t * TN


def _tile_nodes(t):
    return min(TN, SK_LOC - t * TN)


# groups of up to 4 tiles for the MLP/aggregation stage
GROUPS = []
_t = 0
while _t < NT:
    ts = list(range(_t, min(_t + 4, NT)))
    GROUPS.append(ts)
    _t += 4

# ---- tuning knobs ----
RELU_PAT = "A"
Y1_PAT = "A"
OUT_PAT = "D"
COPY_PAT = "D"
S_CHUNKS = 8
X_CHUNK = 512
PSUM_M = 3
PSUM_Z = 2
PSUM_Y = 2
PSUM_T = 1


def _host_preprocess(inputs):
    x_tokens = np.asarray(inputs["x_tokens"]).astype(np.int64)
    edge_tokens = np.asarray(inputs["edge_tokens"]).astype(np.int64)
    intra_ei = np.asarray(inputs["intra_ei"]).astype(np.int64)
    node_ids = np.asarray(inputs["node_ids"]).astype(np.int64)
    valid = np.asarray(inputs["valid"]).astype(bool)
    log_probs = np.asarray(inputs["log_probs"]).astype(np.float32)
    batch_graph = np.asarray(inputs["batch_graph"]).astype(np.int64)

    src, dst = intra_ei[0], intra_ei[1]
    e_sub = src // K_NODES
    assert np.array_equal(dst // K_NODES, e_sub), "edges must be intra-subgraph"

    core_of_e = e_sub // S_LOC
    sub_loc_e = e_sub % S_LOC
    tile_of_e = sub_loc_e // SG_T
    key = core_of_e * NT + tile_of_e
    counts = np.bincount(key, minlength=NCORES * NT)
    assert counts.max() <= E_CAP, f"edge overflow: {counts.max()} > {E_CAP}"

    order = np.argsort(key, kind="stable")
    starts = np.zeros(NCORES * NT, dtype=np.int64)
    starts[1:] = np.cumsum(counts)[:-1]
    slot = np.empty(E_ALL, dtype=np.int64)
    slot[order] = np.arange(E_ALL) - starts[key[order]]

    ec = NT * E_CAP                       # merged s/bond one-hot columns
    ecd = NT * D_W                        # d one-hot columns
    j_of = lambda s: s                    # clarity
    src_loc = (src % SK_LOC) - tile_of_e * TN
    dst_loc = (dst % SK_LOC) - tile_of_e * TN
    col = tile_of_e * E_CAP + slot
    chunk = slot // 128
    e_loc = slot % 128
    dcol = tile_of_e * D_W + chunk * TN + dst_loc

    SB_oh = np.zeros((NCORES, 128, ec), dtype=F8)
    SB_oh[core_of_e, src_loc, col] = valid[src].astype(F8)
    # bond rows live at partitions 120..127 of the same stationary
    SB_oh[core_of_e, 120 + edge_tokens, col] = np.asarray(1, dtype=F8)
    # the dst one-hot is built on device from a compact index map:
    # didx[slot%128, tile*2+chunk] = dst_loc (255 = empty slot)
    didx = np.full((NCORES, 128, NT * 2), 255.0, dtype=BF16)
    didx[core_of_e, e_loc, tile_of_e * 2 + chunk] = dst_loc.astype(BF16)
    colio8 = np.tile(np.arange(TN, dtype=np.float32), 8).reshape(1, 8 * TN)
    colio8 = colio8.repeat(128, 0).astype(BF16)

    j = np.arange(SK_ALL)
    j_core = j // SK_LOC
    j_loc = j % SK_LOC
    Xoh = np.zeros((NCORES, 128, SK_LOC), dtype=F8)
    Xoh[j_core, x_tokens, j_loc] = np.asarray(1, dtype=F8)

    # Subgraph pooling one-hot: node row within tile -> global subgraph col
    vm = node_ids >= 0
    tile_of_j = j_loc // TN
    row_of_j = j_loc - tile_of_j * TN
    P1 = np.zeros((NCORES, 128, S_LOC), dtype=BF16)
    P1[j_core, row_of_j, j_loc // K_NODES] = vm.astype(BF16)
    cnt = np.bincount(j // K_NODES, weights=vm.astype(np.float64), minlength=S_ALL)
    recip_cnt = (1.0 / np.maximum(cnt, 1.0)).astype(np.float32).reshape(NCORES, 1, S_LOC)

    n = np.arange(N_TOTAL)
    Gmat = np.zeros((NCORES, 128, NQ * NUM_GRAPHS), dtype=BF16)
    Gmat[n // NCAN_LOC, n % 128, ((n % NCAN_LOC) // 128) * NUM_GRAPHS + batch_graph] = (
        np.asarray(1, dtype=BF16)
    )

    lp = log_probs.reshape(NCORES, 1, S_LOC).astype(np.float32)

    atom_emb = np.asarray(inputs["atom_emb"]).astype(np.float32)
    role_emb = np.asarray(inputs["role_emb"]).astype(np.float32)
    # reference: role = role_emb[is_root] -> roots get row 1, others row 0
    atom2 = np.zeros((128, H), dtype=BF16)
    atom2[:IN_CH] = (atom_emb + role_emb[0]).astype(BF16)
    diff = (role_emb[1] - role_emb[0]).reshape(1, H).astype(BF16)
    rootmask = (np.arange(X_CHUNK) % K_NODES == 0).reshape(1, X_CHUNK).astype(BF16)

    bond = np.asarray(inputs["bond_emb"]).astype(BF16)      # [8, H]
    bond_tiled = np.tile(bond[:, None, :], (1, NT, 1)).reshape(8, NT * H)

    w1 = np.asarray(inputs["mlp_w1"]).astype(BF16)
    w2 = np.asarray(inputs["mlp_w2"]).astype(BF16)
    wpack = np.concatenate(
        [w1.transpose(1, 0, 2).reshape(H, L_LAYERS * H),
         w2.transpose(1, 0, 2).reshape(H, L_LAYERS * H)], axis=1
    )
    bpack = np.concatenate(
        [np.asarray(inputs["mlp_b1"]).astype(np.float32).T,
         np.asarray(inputs["mlp_b2"]).astype(np.float32).T], axis=1
    )

    per_core = []
    for c in range(NCORES):
        per_core.append(
            {
                "sb_oh": np.ascontiguousarray(SB_oh[c]),
                "didx": np.ascontiguousarray(didx[c]),
                "xoh": np.ascontiguousarray(Xoh[c]),
                "p1": np.ascontiguousarray(P1[c]),
                "gmat": np.ascontiguousarray(Gmat[c]),
                "recip_cnt": np.ascontiguousarray(recip_cnt[c]),
                "lp": np.ascontiguousarray(lp[c]),
            }
        )

    shared = {
        "colio8": colio8,
        "atom2": atom2,
        "diff": diff,
        "rootmask": rootmask,
        "bond_tiled": np.ascontiguousarray(bond_tiled),
        "wpack": np.ascontiguousarray(wpack),
        "bpack": np.ascontiguousarray(bpack),
        "eps": np.asarray(inputs["eps"]).astype(np.float32).reshape(1, L_LAYERS),
        "alpha": np.asarray(inputs["ht_alpha"]).astype(np.float32).reshape(1, 1),
        "ones128": np.ones((1, 128), dtype=np.float32),
        "ident": np.eye(128, dtype=BF16),
    }
    return per_core, shared


def _build_bass(repeat=1):
    import concourse.bass as bass
    import concourse.mybir as mybir
    from concourse import bacc
    from concourse.tile import TileContext

    f32 = mybir.dt.float32
    bf16 = mybir.dt.bfloat16
    fp8 = mybir.dt.float8e4
    AF = mybir.ActivationFunctionType
    ALU = mybir.AluOpType
    AX = mybir.AxisListType

    ec = NT * E_CAP
    ecd = NT * D_W

    nc = bacc.Bacc("TRN2", target_bir_lowering=False, debug=False, num_devices=NCORES)

    def din(name, shape, dt):
        return nc.dram_tensor(name, shape, dt, kind="ExternalInput").ap()

    sb_d = din("sb_oh", [128, ec], fp8)
    di_d = din("didx", [128, NT * 2], bf16)
    cio_d = din("colio8", [128, 8 * TN], bf16)
    x_d = din("xoh", [128, SK_LOC], fp8)
    p1_d = din("p1", [128, S_LOC], bf16)
    g_d = din("gmat", [128, NQ * NUM_GRAPHS], bf16)
    rc_d = din("recip_cnt", [1, S_LOC], f32)
    lp_d = din("lp", [1, S_LOC], f32)
    atom_d = din("atom2", [128, H], bf16)
    diff_d = din("diff", [1, H], bf16)
    rm_d = din("rootmask", [1, X_CHUNK], bf16)
    bt_d = din("bond_tiled", [8, NT * H], bf16)
    wp_d = din("wpack", [128, 2 * L_LAYERS * H], bf16)
    bp_d = din("bpack", [128, 2 * L_LAYERS], f32)
    eps_d = din("eps", [1, L_LAYERS], f32)
    al_d = din("alpha", [1, 1], f32)
    ones_d = din("ones128", [1, 128], f32)
    id_d = din("ident", [128, 128], bf16)

    out_d = nc.dram_tensor("out", [NUM_GRAPHS, H], f32, kind="ExternalOutput").ap()

    def _kernel_body(tc):
        with tc.tile_pool(name="persist", bufs=1) as pp:
            sb_sb = pp.tile([128, ec], fp8, tag="s")
            d_sb = pp.tile([128, ecd], fp8, tag="d")
            di_sb = pp.tile([128, NT * 2], bf16, tag="di")
            cio_sb = pp.tile([128, 8 * TN], bf16, tag="cio")
            hT = pp.tile([128, SK_LOC], bf16, tag="hT")
            h_nm = pp.tile([128, NT * 128], bf16, tag="hnm")
            p1_sb = pp.tile([128, S_LOC], bf16, tag="p1")
            g_sb = pp.tile([128, NQ * NUM_GRAPHS], bf16, tag="g")
            atom_sb = pp.tile([128, H], bf16, tag="atom")
            diff_sb = pp.tile([1, H], bf16, tag="diff")
            rm_sb = pp.tile([1, X_CHUNK], bf16, tag="rm")
            wp_sb = pp.tile([128, 2 * L_LAYERS * H], bf16, tag="wp")
            bp_sb = pp.tile([128, 2 * L_LAYERS], f32, tag="bp")
            eps_sb = pp.tile([1, L_LAYERS], f32, tag="eps")
            e1bc = pp.tile([128, L_LAYERS], f32, tag="e1bc")
            al_sb = pp.tile([1, 1], f32, tag="al")
            ones_sb = pp.tile([1, 128], f32, tag="ones")
            id_sb = pp.tile([128, 128], bf16, tag="id")
            w_bc = pp.tile([128, S_LOC], f32, tag="wbc")
            rbc = pp.tile([128, S_LOC // M_SUB], f32, tag="rbc")
            ndT = pp.tile([128, NCAN_LOC], f32, tag="ndT")

            nc.gpsimd.dma_start(out=wp_sb, in_=wp_d)
            nc.gpsimd.dma_start(out=bp_sb, in_=bp_d)
            nc.gpsimd.dma_start(out=eps_sb, in_=eps_d)
            nc.gpsimd.dma_start(out=ones_sb, in_=ones_d)
            nc.gpsimd.dma_start(out=id_sb, in_=id_d)
            # bond rows of every h_nm block, written once
            nc.gpsimd.dma_start(out=h_nm[120:128, :NT * H], in_=bt_d)
            # rows 64..119 of the short last tile are never written by the
            # transposes; zero them so stray NaNs can't leak through the
            # (zero-weighted) gather/pool contractions
            if _tile_nodes(NT - 1) < TN:
                nc.gpsimd.memset(
                    h_nm[_tile_nodes(NT - 1) : TN, (NT - 1) * 128 : NT * 128], 0
                )
            nc.gpsimd.dma_start(out=di_sb, in_=di_d)
            nc.gpsimd.dma_start(out=cio_sb, in_=cio_d)
            # build the dst one-hot on device: one is_equal per 8-chunk swath
            nch_all = NT * 2
            sw0 = 0
            while sw0 < nch_all:
                K = min(8, nch_all - sw0)
                nc.vector.tensor_tensor(
                    d_sb[:, sw0 * TN : (sw0 + K) * TN].rearrange(
                        "p (a b) -> p a b", b=TN
                    ),
                    cio_sb[:, : K * TN].rearrange("p (a b) -> p a b", b=TN),
                    di_sb[:, sw0 : sw0 + K].broadcast_to([128, K, TN]),
                    ALU.is_equal,
                )
                sw0 += K
            sch = ec // S_CHUNKS
            for i in range(S_CHUNKS):
                nc.gpsimd.dma_start(
                    out=sb_sb[:, i * sch : (i + 1) * sch],
                    in_=sb_d[:, i * sch : (i + 1) * sch],
                )
            nc.gpsimd.dma_start(out=p1_sb, in_=p1_d)
            nc.gpsimd.dma_start(out=g_sb, in_=g_d)
            nc.gpsimd.dma_start(out=al_sb, in_=al_d)
            nc.sync.dma_start(out=atom_sb, in_=atom_d)
            nc.sync.dma_start(out=diff_sb, in_=diff_d)
            nc.sync.dma_start(out=rm_sb, in_=rm_d)

            # ---------------- embed ----------------
            with (
                tc.tile_pool(name="emb_sb", bufs=3) as ep,
                tc.tile_pool(name="sm_sb", bufs=1) as smp,
                tc.tile_pool(name="emb_ps", bufs=3, space="PSUM") as epp,
                tc.tile_pool(name="emb_ps1", bufs=1, space="PSUM") as epp1,
                tc.tile_pool(name="emb_ptr", bufs=2, space="PSUM") as eptr,
            ):
                pse = epp1.tile([128, L_LAYERS], f32, tag="pse")
                nc.tensor.matmul(pse, lhsT=ones_sb, rhs=eps_sb, start=True, stop=True)
                nc.scalar.activation(e1bc, pse, AF.Copy, bias=1.0)
                rc_sb = smp.tile([1, S_LOC], f32, tag="rc")
                lp_sb = smp.tile([1, S_LOC], f32, tag="lp")
                nc.gpsimd.dma_start(out=rc_sb, in_=rc_d)
                nc.gpsimd.dma_start(out=lp_sb, in_=lp_d)
                nc.vector.tensor_scalar(
                    lp_sb, lp_sb, al_sb[:, 0:1], -1.0, op0=ALU.mult, op1=ALU.mult
                )
                nc.scalar.activation(lp_sb, lp_sb, AF.Exp)
                et = lp_sb
                s4 = smp.tile([1, S_LOC // M_SUB], f32, tag="s4")
                nc.vector.tensor_reduce(
                    s4, et.rearrange("p (a b) -> p a b", b=M_SUB), AX.X, ALU.add
                )
                r4 = smp.tile([1, S_LOC // M_SUB], f32, tag="r4")
                nc.vector.reciprocal(r4, s4)
                nc.vector.tensor_tensor(et, et, rc_sb, ALU.mult)
                wr = et
                for q in range(S_LOC // 512):
                    pw = epp1.tile([128, 512], f32, tag="pw")
                    nc.tensor.matmul(
                        pw, lhsT=ones_sb, rhs=wr[:, q * 512 : (q + 1) * 512],
                        start=True, stop=True,
                    )
                    nc.vector.tensor_copy(w_bc[:, q * 512 : (q + 1) * 512], pw)
                pw = epp1.tile([128, 512], f32, tag="pw")
                nc.tensor.matmul(pw, lhsT=ones_sb, rhs=r4, start=True, stop=True)
                nc.vector.tensor_copy(rbc, pw[:, : S_LOC // M_SUB])

                for q in range(SK_LOC // X_CHUNK):
                    qsl = slice(q * X_CHUNK, (q + 1) * X_CHUNK)
                    xt = ep.tile([128, X_CHUNK], fp8, tag="x")
                    nc.sync.dma_start(out=xt, in_=x_d[:, qsl])
                    ps = epp.tile([128, X_CHUNK], f32, tag="ps")
                    nc.tensor.matmul(ps, lhsT=atom_sb, rhs=xt, start=True, stop=False)
                    nc.tensor.matmul(ps, lhsT=diff_sb, rhs=rm_sb, start=False, stop=True)
                    nc.scalar.activation(hT[:, qsl], ps, AF.Copy)
                # hT -> h_nm blocks (120-node tiles) per 4-tile group
                for gi, ts in enumerate(GROUPS):
                    pn = max(_tile_nodes(t) for t in ts)
                    ptr = eptr.tile([128, len(ts) * 128], bf16, tag="ptr")
                    for k, t in enumerate(ts):
                        nb, n0 = _tile_nodes(t), _tile_base(t)
                        nc.tensor.transpose(
                            ptr[0:nb, k * 128 : (k + 1) * 128],
                            hT[:, n0 : n0 + nb],
                            id_sb,
                        )
                    nc.vector.tensor_copy(
                        h_nm[0:pn, ts[0] * 128 : ts[0] * 128 + len(ts) * 128],
                        ptr[0:pn],
                    )

            # ---------------- layers ----------------
            with (
                tc.tile_pool(name="msg_sb", bufs=3) as mp,
                tc.tile_pool(name="zy_sb", bufs=3) as zp,
                tc.tile_pool(name="ps_m", bufs=PSUM_M, space="PSUM") as pm,
                tc.tile_pool(name="ps_z", bufs=PSUM_Z, space="PSUM") as pz,
                tc.tile_pool(name="ps_mlp", bufs=PSUM_Y, space="PSUM") as pmlp,
                tc.tile_pool(name="ps_tr", bufs=PSUM_T, space="PSUM") as ptp,
            ):
                for l in range(L_LAYERS):
                    w1_l = wp_sb[:, l * H : (l + 1) * H]
                    w2_l = wp_sb[:, (L_LAYERS + l) * H : (L_LAYERS + l + 1) * H]
                    b1_l = bp_sb[:, l : l + 1]
                    b2_l = bp_sb[:, L_LAYERS + l : L_LAYERS + l + 1]
                    for gidx, ts in enumerate(GROUPS):
                        n0 = _tile_base(ts[0])
                        gw = sum(_tile_nodes(t) for t in ts)
                        gsl = slice(n0, n0 + gw)
                        psz = pz.tile([128, 480], f32, tag="z")
                        # pairs of tiles: merged gather+bond, then relu, then
                        # scatter (gathers of both pairs emitted first)
                        pairs = [ts[i : i + 2] for i in range(0, len(ts), 2)]
                        msgs = []
                        for pi, pts in enumerate(pairs):
                            psm = pm.tile([128, 512], f32, tag="m")
                            for k, t in enumerate(pts):
                                for ch in range(2):
                                    c0 = t * E_CAP + ch * 128
                                    osl = slice((2 * k + ch) * 128, (2 * k + ch + 1) * 128)
                                    nc.tensor.matmul(
                                        psm[:, osl],
                                        lhsT=sb_sb[:, c0 : c0 + 128],
                                        rhs=h_nm[:, t * 128 : (t + 1) * 128],
                                        start=True,
                                        stop=True,
                                    )
                            msg = mp.tile([128, 512], bf16, tag="msg")
                            msgs.append(msg)
                            eng = RELU_PAT[(gidx * 2 + pi) % len(RELU_PAT)]
                            w = len(pts) * 256
                            if eng == "A":
                                nc.scalar.activation(msg[:, :w], psm[:, :w], AF.Relu)
                            else:
                                nc.vector.tensor_scalar_max(msg[:, :w], psm[:, :w], 0.0)
                        for pi, pts in enumerate(pairs):
                            msg = msgs[pi]
                            for k, t in enumerate(pts):
                                tl = 2 * pi + k
                                nb = _tile_nodes(t)
                                for ch in range(2):
                                    dc0 = t * D_W + ch * TN
                                    nc.tensor.matmul(
                                        psz[:, tl * TN : tl * TN + nb],
                                        lhsT=msg[:, (2 * k + ch) * 128 : (2 * k + ch + 1) * 128],
                                        rhs=d_sb[:, dc0 : dc0 + nb],
                                        start=(ch == 0),
                                        stop=(ch == 1),
                                    )
                        zin = zp.tile([128, 480], bf16, tag="zin")
                        nc.vector.scalar_tensor_tensor(
                            zin[:, :gw], hT[:, gsl], e1bc[:, l : l + 1], psz[:, :gw],
                            op0=ALU.mult, op1=ALU.add,
                        )
                        psy = pmlp.tile([128, 480], f32, tag="y")
                        nc.tensor.matmul(
                            psy[:, :gw], lhsT=w1_l, rhs=zin[:, :gw], start=True, stop=True
                        )
                        y1 = zp.tile([128, 480], bf16, tag="y1")
                        if Y1_PAT[gidx % len(Y1_PAT)] == "A":
                            nc.scalar.activation(y1[:, :gw], psy[:, :gw], AF.Relu, bias=b1_l)
                        else:
                            nc.vector.tensor_scalar(
                                y1[:, :gw], psy[:, :gw], b1_l, 0.0, op0=ALU.add, op1=ALU.max
                            )
                        psz2 = pmlp.tile([128, 480], f32, tag="y")
                        nc.tensor.matmul(
                            psz2[:, :gw], lhsT=w2_l, rhs=y1[:, :gw], start=True, stop=True
                        )
                        if OUT_PAT[gidx % len(OUT_PAT)] == "A":
                            nc.scalar.activation(hT[:, gsl], psz2[:, :gw], AF.Identity, bias=b2_l)
                        else:
                            nc.vector.tensor_scalar(
                                hT[:, gsl], psz2[:, :gw], b2_l, None, op0=ALU.add
                            )
                        # hT -> h_nm transposes (PE + one copy per group)
                        pn = max(_tile_nodes(t) for t in ts)
                        ptr = ptp.tile([128, len(ts) * 128], bf16, tag="tr")
                        for k, t in enumerate(ts):
                            nb, tb = _tile_nodes(t), _tile_base(t)
                            nc.tensor.transpose(
                                ptr[0:nb, k * 128 : (k + 1) * 128],
                                hT[:, tb : tb + nb],
                                id_sb,
                            )
                        ceng = COPY_PAT[gidx % len(COPY_PAT)]
                        dst = h_nm[0:pn, ts[0] * 128 : ts[0] * 128 + len(ts) * 128]
                        if ceng == "A":
                            nc.scalar.activation(dst, ptr[0:pn], AF.Copy)
                        else:
                            nc.vector.tensor_copy(dst, ptr[0:pn])

            # ---------------- pooling ----------------
            with (
                tc.tile_pool(name="po_sb", bufs=1) as po,
                tc.tile_pool(name="ps_hs", bufs=1, space="PSUM") as phs,
                tc.tile_pool(name="ps_sm1", bufs=1, space="PSUM") as psm_q,
                tc.tile_pool(name="ps_o", bufs=1, space="PSUM") as pso,
            ):
                hs = phs.tile([128, S_LOC], f32, tag="hs")
                for t in range(NT):
                    s0 = t * SG_T
                    sw = min(SG_T, S_LOC - s0)
                    nc.tensor.matmul(
                        hs[:, s0 : s0 + sw],
                        lhsT=h_nm[:, t * 128 : (t + 1) * 128],
                        rhs=p1_sb[:, s0 : s0 + sw],
                        start=True,
                        stop=True,
                    )
                wt = w_bc
                nc.vector.tensor_tensor(wt, hs, w_bc, ALU.mult)
                nc.vector.tensor_reduce(
                    ndT,
                    wt.rearrange("p (a b) -> p a b", b=M_SUB),
                    AX.X,
                    ALU.add,
                )
                ndTb = po.tile([128, NCAN_LOC], bf16, tag="ndTb")
                nc.vector.tensor_tensor(ndTb, ndT, rbc, ALU.mult)
                pout = pso.tile([NUM_GRAPHS, H], f32, tag="po")
                for q in range(NQ):
                    ptq = psm_q.tile([128, 128], bf16, tag="pq")
                    nc.tensor.transpose(ptq, ndTb[:, q * 128 : (q + 1) * 128], id_sb)
                    nnm = po.tile([128, 128], bf16, tag="nnm")
                    nc.vector.tensor_copy(nnm, ptq)
                    nc.tensor.matmul(
                        pout,
                        lhsT=g_sb[:, q * NUM_GRAPHS : (q + 1) * NUM_GRAPHS],
                        rhs=nnm,
                        start=(q == 0),
                        stop=(q == NQ - 1),
                    )
                outs = po.tile([NUM_GRAPHS, H], f32, tag="outs")
                nc.scalar.activation(outs, pout, AF.Copy)
                nc.sync.dma_start(out=out_d, in_=outs)

    with TileContext(nc) as tc:
        if repeat > 1:
            with tc.For_i(0, repeat, 1) as _i:
                _kernel_body(tc)
        else:
            _kernel_body(tc)

    nc.finalize()
    return nc


_CACHE = {}


def _get_bass():
    if "nc" not in _CACHE:
        _CACHE["nc"] = _build_bass()
    return _CACHE["nc"]


def kernel(**inputs):
    from concourse.bass_utils import run_bass_kernel_spmd

    per_core, shared = _host_preprocess(inputs)
    in_maps = [{**pc, **shared} for pc in per_core]
    nc = _get_bass()
    res = run_bass_kernel_spmd(nc, in_maps, core_ids=list(range(NCORES)))
    out = np.zeros((NUM_GRAPHS, H), dtype=np.float32)
    for r in res.results:
        out += np.asarray(r["out"], dtype=np.float32)
    return out


# revision 4
# speedup vs baseline: 3.0357x; 3.0357x over previous
"""Trainium2 Bass kernel for nn_Arch7V3GraphEncoder (gnn_message_passing), v5.

v2 + merged gather/bond matmuls: tiles hold 120 nodes (15 subgraphs), and
partitions 120-127 of every h_nm block carry the bond embedding table, so a
single one-hot stationary [128, slots] per chunk computes
h[src]*valid + bond_emb[tok] in one matmul (half the PE instructions of the
separate gather+bond pair). Subgraph pooling columns are globally ordered, so
the 15-subgraph tiling leaves the HT-softmax reduce untouched.
"""

import sys

sys.path.insert(0, "/opt/trn_rl_repo")

import numpy as np
import ml_dtypes

BF16 = ml_dtypes.bfloat16
F8 = ml_dtypes.float8_e4m3

# Problem constants (hardcoded per spec).
N_TOTAL = 4096
M_SUB = 4
K_NODES = 8
L_LAYERS = 4
H = 128
NUM_GRAPHS = 32
IN_CH = 119
EDGE_DIM = 8
S_ALL = N_TOTAL * M_SUB          # 16384 subgraphs
SK_ALL = S_ALL * K_NODES         # 131072 flat nodes
E_ALL = 12 * S_ALL               # 196608 edges
NCORES = 8
S_LOC = S_ALL // NCORES          # 2048 subgraphs / core
SK_LOC = SK_ALL // NCORES        # 16384 flat nodes / core
NCAN_LOC = N_TOTAL // NCORES     # 512 canonical nodes / core
NQ = NCAN_LOC // 128             # 4 canonical chunks of 128
E_CAP = 256                      # edge slots per tile (2 chunks of 128)

SG_T = 15                        # subgraphs per tile
TN = SG_T * K_NODES              # 120 nodes per tile
NT = (S_LOC + SG_T - 1) // SG_T  # 137 tiles (last tile has 8 subgraphs)
D_W = 2 * TN                     # d_oh columns per tile (2 chunks x 120)

# node-column base and node count of tile t
def _tile_base(t):
    return t * TN


def _tile_nodes(t):
    return min(TN, SK_LOC - t * TN)


# groups of up to 4 tiles for the MLP/aggregation stage
GROUPS = []
_t = 0
while _t < NT:
    ts = list(range(_t, min(_t + 4, NT)))
    GROUPS.append(ts)
    _t += 4

# ---- tuning knobs ----
RELU_PAT = "A"
Y1_PAT = "A"
OUT_PAT = "D"
COPY_PAT = "D"
S_CHUNKS = 8
X_CHUNK = 512
PSUM_M = 3
PSUM_Z = 2
PSUM_Y = 2
PSUM_T = 1
BULK_ENG = "S"       # S=SP HWDGE, G=gpsimd SWDGE for bulk input loads


def _host_preprocess(inputs):
    x_tokens = np.asarray(inputs["x_tokens"]).astype(np.int64)
    edge_tokens = np.asarray(inputs["edge_tokens"]).astype(np.int64)
    intra_ei = np.asarray(inputs["intra_ei"]).astype(np.int64)
    node_ids = np.asarray(inputs["node_ids"]).astype(np.int64)
    valid = np.asarray(inputs["valid"]).astype(bool)
    log_probs = np.asarray(inputs["log_probs"]).astype(np.float32)
    batch_graph = np.asarray(inputs["batch_graph"]).astype(np.int64)

    src, dst = intra_ei[0], intra_ei[1]
    e_sub = src // K_NODES
    assert np.array_equal(dst // K_NODES, e_sub), "edges must be intra-subgraph"

    core_of_e = e_sub // S_LOC
    sub_loc_e = e_sub % S_LOC
    tile_of_e = sub_loc_e // SG_T
    key = core_of_e * NT + tile_of_e
    counts = np.bincount(key, minlength=NCORES * NT)
    assert counts.max() <= E_CAP, f"edge overflow: {counts.max()} > {E_CAP}"

    order = np.argsort(key, kind="stable")
    starts = np.zeros(NCORES * NT, dtype=np.int64)
    starts[1:] = np.cumsum(counts)[:-1]
    slot = np.empty(E_ALL, dtype=np.int64)
    slot[order] = np.arange(E_ALL) - starts[key[order]]

    ec = NT * E_CAP                       # merged s/bond one-hot columns
    ecd = NT * D_W                        # d one-hot columns
    j_of = lambda s: s                    # clarity
    src_loc = (src % SK_LOC) - tile_of_e * TN
    dst_loc = (dst % SK_LOC) - tile_of_e * TN
    col = tile_of_e * E_CAP + slot
    chunk = slot // 128
    e_loc = slot % 128
    dcol = tile_of_e * D_W + chunk * TN + dst_loc

    SB_oh = np.zeros((NCORES, 128, ec), dtype=F8)
    SB_oh[core_of_e, src_loc, col] = valid[src].astype(F8)
    # bond rows live at partitions 120..127 of the same stationary
    SB_oh[core_of_e, 120 + edge_tokens, col] = np.asarray(1, dtype=F8)
    # the dst one-hot is built on device from a compact index map:
    # didx[slot%128, tile*2+chunk] = dst_loc (255 = empty slot)
    didx = np.full((NCORES, 128, NT * 2), 255.0, dtype=BF16)
    didx[core_of_e, e_loc, tile_of_e * 2 + chunk] = dst_loc.astype(BF16)
    colio8 = np.tile(np.arange(TN, dtype=np.float32), 8).reshape(1, 8 * TN)
    colio8 = colio8.repeat(128, 0).astype(BF16)

    j = np.arange(SK_ALL)
    j_core = j // SK_LOC
    j_loc = j % SK_LOC
    Xoh = np.zeros((NCORES, 128, SK_LOC), dtype=F8)
    Xoh[j_core, x_tokens, j_loc] = np.asarray(1, dtype=F8)

    # Subgraph pooling one-hot: node row within tile -> global subgraph col
    vm = node_ids >= 0
    tile_of_j = j_loc // TN
    row_of_j = j_loc - tile_of_j * TN
    P1 = np.zeros((NCORES, 128, S_LOC), dtype=BF16)
    P1[j_core, row_of_j, j_loc // K_NODES] = vm.astype(BF16)
    cnt = np.bincount(j // K_NODES, weights=vm.astype(np.float64), minlength=S_ALL)
    recip_cnt = (1.0 / np.maximum(cnt, 1.0)).astype(np.float32).reshape(NCORES, 1, S_LOC)

    n = np.arange(N_TOTAL)
    Gmat = np.zeros((NCORES, 128, NQ * NUM_GRAPHS), dtype=BF16)
    Gmat[n // NCAN_LOC, n % 128, ((n % NCAN_LOC) // 128) * NUM_GRAPHS + batch_graph] = (
        np.asarray(1, dtype=BF16)
    )

    lp = log_probs.reshape(NCORES, 1, S_LOC).astype(np.float32)

    atom_emb = np.asarray(inputs["atom_emb"]).astype(np.float32)
    role_emb = np.asarray(inputs["role_emb"]).astype(np.float32)
    # reference: role = role_emb[is_root] -> roots get row 1, others row 0
    atom2 = np.zeros((128, H), dtype=BF16)
    atom2[:IN_CH] = (atom_emb + role_emb[0]).astype(BF16)
    diff = (role_emb[1] - role_emb[0]).reshape(1, H).astype(BF16)
    rootmask = (np.arange(X_CHUNK) % K_NODES == 0).reshape(1, X_CHUNK).astype(BF16)

    bond = np.asarray(inputs["bond_emb"]).astype(BF16)      # [8, H]
    bond_tiled = np.tile(bond[:, None, :], (1, NT, 1)).reshape(8, NT * H)

    w1 = np.asarray(inputs["mlp_w1"]).astype(BF16)
    w2 = np.asarray(inputs["mlp_w2"]).astype(BF16)
    wpack = np.concatenate(
        [w1.transpose(1, 0, 2).reshape(H, L_LAYERS * H),
         w2.transpose(1, 0, 2).reshape(H, L_LAYERS * H)], axis=1
    )
    bpack = np.concatenate(
        [np.asarray(inputs["mlp_b1"]).astype(np.float32).T,
         np.asarray(inputs["mlp_b2"]).astype(np.float32).T], axis=1
    )

    per_core = []
    for c in range(NCORES):
        per_core.append(
            {
                "sb_oh": np.ascontiguousarray(SB_oh[c]),
                "didx": np.ascontiguousarray(didx[c]),
                "xoh": np.ascontiguousarray(Xoh[c]),
                "p1": np.ascontiguousarray(P1[c]),
                "gmat": np.ascontiguousarray(Gmat[c]),
                "recip_cnt": np.ascontiguousarray(recip_cnt[c]),
                "lp": np.ascontiguousarray(lp[c]),
            }
        )

    shared = {
        "colio8": colio8,
        "atom2": atom2,
        "diff": diff,
        "rootmask": rootmask,
        "bond_tiled": np.ascontiguousarray(bond_tiled),
        "wpack": np.ascontiguousarray(wpack),
        "bpack": np.ascontiguousarray(bpack),
        "eps": np.asarray(inputs["eps"]).astype(np.float32).reshape(1, L_LAYERS),
        "alpha": np.asarray(inputs["ht_alpha"]).astype(np.float32).reshape(1, 1),
        "ones128": np.ones((1, 128), dtype=np.float32),
        "ident": np.eye(128, dtype=BF16),
    }
    return per_core, shared


def _build_bass(repeat=1):
    import concourse.bass as bass
    import concourse.mybir as mybir
    from concourse import bacc
    from concourse.tile import TileContext

    f32 = mybir.dt.float32
    bf16 = mybir.dt.bfloat16
    fp8 = mybir.dt.float8e4
    AF = mybir.ActivationFunctionType
    ALU = mybir.AluOpType
    AX = mybir.AxisListType

    ec = NT * E_CAP
    ecd = NT * D_W

    nc = bacc.Bacc("TRN2", target_bir_lowering=False, debug=False, num_devices=NCORES)

    def din(name, shape, dt):
        return nc.dram_tensor(name, shape, dt, kind="ExternalInput").ap()

    sb_d = din("sb_oh", [128, ec], fp8)
    di_d = din("didx", [128, NT * 2], bf16)
    cio_d = din("colio8", [128, 8 * TN], bf16)
    x_d = din("xoh", [128, SK_LOC], fp8)
    p1_d = din("p1", [128, S_LOC], bf16)
    g_d = din("gmat", [128, NQ * NUM_GRAPHS], bf16)
    rc_d = din("recip_cnt", [1, S_LOC], f32)
    lp_d = din("lp", [1, S_LOC], f32)
    atom_d = din("atom2", [128, H], bf16)
    diff_d = din("diff", [1, H], bf16)
    rm_d = din("rootmask", [1, X_CHUNK], bf16)
    bt_d = din("bond_tiled", [8, NT * H], bf16)
    wp_d = din("wpack", [128, 2 * L_LAYERS * H], bf16)
    bp_d = din("bpack", [128, 2 * L_LAYERS], f32)
    eps_d = din("eps", [1, L_LAYERS], f32)
    al_d = din("alpha", [1, 1], f32)
    ones_d = din("ones128", [1, 128], f32)
    id_d = din("ident", [128, 128], bf16)

    out_d = nc.dram_tensor("out", [NUM_GRAPHS, H], f32, kind="ExternalOutput").ap()

    def _kernel_body(tc):
        bulk = nc.sync if BULK_ENG == "S" else nc.gpsimd
        with tc.tile_pool(name="persist", bufs=1) as pp:
            sb_sb = pp.tile([128, ec], fp8, tag="s")
            d_sb = pp.tile([128, ecd], fp8, tag="d")
            di_sb = pp.tile([128, NT * 2], bf16, tag="di")
            cio_sb = pp.tile([128, 8 * TN], bf16, tag="cio")
            hT = pp.tile([128, SK_LOC], bf16, tag="hT")
            h_nm = pp.tile([128, NT * 128], bf16, tag="hnm")
            p1_sb = pp.tile([128, S_LOC], bf16, tag="p1")
            g_sb = pp.tile([128, NQ * NUM_GRAPHS], bf16, tag="g")
            atom_sb = pp.tile([128, H], bf16, tag="atom")
            diff_sb = pp.tile([1, H], bf16, tag="diff")
            rm_sb = pp.tile([1, X_CHUNK], bf16, tag="rm")
            wp_sb = pp.tile([128, 2 * L_LAYERS * H], bf16, tag="wp")
            bp_sb = pp.tile([128, 2 * L_LAYERS], f32, tag="bp")
            eps_sb = pp.tile([1, L_LAYERS], f32, tag="eps")
            e1bc = pp.tile([128, L_LAYERS], f32, tag="e1bc")
            al_sb = pp.tile([1, 1], f32, tag="al")
            ones_sb = pp.tile([1, 128], f32, tag="ones")
            id_sb = pp.tile([128, 128], bf16, tag="id")
            w_bc = pp.tile([128, S_LOC], f32, tag="wbc")
            rbc = pp.tile([128, S_LOC // M_SUB], f32, tag="rbc")
            ndT = pp.tile([128, NCAN_LOC], f32, tag="ndT")

            bulk.dma_start(out=wp_sb, in_=wp_d)
            bulk.dma_start(out=bp_sb, in_=bp_d)
            bulk.dma_start(out=eps_sb, in_=eps_d)
            bulk.dma_start(out=ones_sb, in_=ones_d)
            bulk.dma_start(out=id_sb, in_=id_d)
            # bond rows of every h_nm block, written once
            bulk.dma_start(out=h_nm[120:128, :NT * H], in_=bt_d)
            # rows 64..119 of the short last tile are never written by the
            # transposes; zero them so stray NaNs can't leak through the
            # (zero-weighted) gather/pool contractions
            if _tile_nodes(NT - 1) < TN:
                nc.gpsimd.memset(
                    h_nm[_tile_nodes(NT - 1) : TN, (NT - 1) * 128 : NT * 128], 0
                )
            bulk.dma_start(out=di_sb, in_=di_d)
            bulk.dma_start(out=cio_sb, in_=cio_d)
            # build the dst one-hot on device: one is_equal per 8-chunk swath
            nch_all = NT * 2
            sw0 = 0
            while sw0 < nch_all:
                K = min(8, nch_all - sw0)
                nc.vector.tensor_tensor(
                    d_sb[:, sw0 * TN : (sw0 + K) * TN].rearrange(
                        "p (a b) -> p a b", b=TN
                    ),
                    cio_sb[:, : K * TN].rearrange("p (a b) -> p a b", b=TN),
                    di_sb[:, sw0 : sw0 + K].broadcast_to([128, K, TN]),
                    ALU.is_equal,
                )
                sw0 += K
            sch = ec // S_CHUNKS
            for i in range(S_CHUNKS):
                bulk.dma_start(
                    out=sb_sb[:, i * sch : (i + 1) * sch],
                    in_=sb_d[:, i * sch : (i + 1) * sch],
                )
            bulk.dma_start(out=p1_sb, in_=p1_d)
            bulk.dma_start(out=g_sb, in_=g_d)
            bulk.dma_start(out=al_sb, in_=al_d)
            nc.sync.dma_start(out=atom_sb, in_=atom_d)
            nc.sync.dma_start(out=diff_sb, in_=diff_d)
            nc.sync.dma_start(out=rm_sb, in_=rm_d)

            # ---------------- embed ----------------
            with (
                tc.tile_pool(name="emb_sb", bufs=3) as ep,
                tc.tile_pool(name="sm_sb", bufs=1) as smp,
                tc.tile_pool(name="emb_ps", bufs=3, space="PSUM") as epp,
                tc.tile_pool(name="emb_ps1", bufs=1, space="PSUM") as epp1,
                tc.tile_pool(name="emb_ptr", bufs=2, space="PSUM") as eptr,
            ):
                pse = epp1.tile([128, L_LAYERS], f32, tag="pse")
                nc.tensor.matmul(pse, lhsT=ones_sb, rhs=eps_sb, start=True, stop=True)
                nc.scalar.activation(e1bc, pse, AF.Copy, bias=1.0)
                rc_sb = smp.tile([1, S_LOC], f32, tag="rc")
                lp_sb = smp.tile([1, S_LOC], f32, tag="lp")
                bulk.dma_start(out=rc_sb, in_=rc_d)
                bulk.dma_start(out=lp_sb, in_=lp_d)
                nc.vector.tensor_scalar(
                    lp_sb, lp_sb, al_sb[:, 0:1], -1.0, op0=ALU.mult, op1=ALU.mult
                )
                nc.scalar.activation(lp_sb, lp_sb, AF.Exp)
                et = lp_sb
                s4 = smp.tile([1, S_LOC // M_SUB], f32, tag="s4")
                nc.vector.tensor_reduce(
                    s4, et.rearrange("p (a b) -> p a b", b=M_SUB), AX.X, ALU.add
                )
                r4 = smp.tile([1, S_LOC // M_SUB], f32, tag="r4")
                nc.vector.reciprocal(r4, s4)
                nc.vector.tensor_tensor(et, et, rc_sb, ALU.mult)
                wr = et
                for q in range(S_LOC // 512):
                    pw = epp1.tile([128, 512], f32, tag="pw")
                    nc.tensor.matmul(
                        pw, lhsT=ones_sb, rhs=wr[:, q * 512 : (q + 1) * 512],
                        start=True, stop=True,
                    )
                    nc.vector.tensor_copy(w_bc[:, q * 512 : (q + 1) * 512], pw)
                pw = epp1.tile([128, 512], f32, tag="pw")
                nc.tensor.matmul(pw, lhsT=ones_sb, rhs=r4, start=True, stop=True)
                nc.vector.tensor_copy(rbc, pw[:, : S_LOC // M_SUB])

                for q in range(SK_LOC // X_CHUNK):
                    qsl = slice(q * X_CHUNK, (q + 1) * X_CHUNK)
                    xt = ep.tile([128, X_CHUNK], fp8, tag="x")
                    nc.sync.dma_start(out=xt, in_=x_d[:, qsl])
                    ps = epp.tile([128, X_CHUNK], f32, tag="ps")
                    nc.tensor.matmul(ps, lhsT=atom_sb, rhs=xt, start=True, stop=False)
                    nc.tensor.matmul(ps, lhsT=diff_sb, rhs=rm_sb, start=False, stop=True)
                    nc.scalar.activation(hT[:, qsl], ps, AF.Copy)
                # hT -> h_nm blocks (120-node tiles) per 4-tile group
                for gi, ts in enumerate(GROUPS):
                    pn = max(_tile_nodes(t) for t in ts)
                    ptr = eptr.tile([128, len(ts) * 128], bf16, tag="ptr")
                    for k, t in enumerate(ts):
                        nb, n0 = _tile_nodes(t), _tile_base(t)
                        nc.tensor.transpose(
                            ptr[0:nb, k * 128 : (k + 1) * 128],
                            hT[:, n0 : n0 + nb],
                            id_sb,
                        )
                    nc.vector.tensor_copy(
                        h_nm[0:pn, ts[0] * 128 : ts[0] * 128 + len(ts) * 128],
                        ptr[0:pn],
                    )

            # ---------------- layers ----------------
            with (
                tc.tile_pool(name="msg_sb", bufs=3) as mp,
                tc.tile_pool(name="zy_sb", bufs=3) as zp,
                tc.tile_pool(name="ps_m", bufs=PSUM_M, space="PSUM") as pm,
                tc.tile_pool(name="ps_z", bufs=PSUM_Z, space="PSUM") as pz,
                tc.tile_pool(name="ps_mlp", bufs=PSUM_Y, space="PSUM") as pmlp,
                tc.tile_pool(name="ps_tr", bufs=PSUM_T, space="PSUM") as ptp,
            ):
                for l in range(L_LAYERS):
                    w1_l = wp_sb[:, l * H : (l + 1) * H]
                    w2_l = wp_sb[:, (L_LAYERS + l) * H : (L_LAYERS + l + 1) * H]
                    b1_l = bp_sb[:, l : l + 1]
                    b2_l = bp_sb[:, L_LAYERS + l : L_LAYERS + l + 1]
                    for gidx, ts in enumerate(GROUPS):
                        n0 = _tile_base(ts[0])
                        gw = sum(_tile_nodes(t) for t in ts)
                        gsl = slice(n0, n0 + gw)
                        psz = pz.tile([128, 480], f32, tag="z")
                        # pairs of tiles: merged gather+bond, then relu, then
                        # scatter (gathers of both pairs emitted first)
                        pairs = [ts[i : i + 2] for i in range(0, len(ts), 2)]
                        msgs = []
                        for pi, pts in enumerate(pairs):
                            psm = pm.tile([128, 512], f32, tag="m")
                            for k, t in enumerate(pts):
                                for ch in range(2):
                                    c0 = t * E_CAP + ch * 128
                                    osl = slice((2 * k + ch) * 128, (2 * k + ch + 1) * 128)
                                    nc.tensor.matmul(
                                        psm[:, osl],
                                        lhsT=sb_sb[:, c0 : c0 + 128],
                                        rhs=h_nm[:, t * 128 : (t + 1) * 128],
                                        start=True,
                                        stop=True,
                                    )
                            msg = mp.tile([128, 512], bf16, tag="msg")
                            msgs.append(msg)
                            eng = RELU_PAT[(gidx * 2 + pi) % len(RELU_PAT)]
                            w = len(pts) * 256
                            if eng == "A":
                                nc.scalar.activation(msg[:, :w], psm[:, :w], AF.Relu)
                            else:
                                nc.vector.tensor_scalar_max(msg[:, :w], psm[:, :w], 0.0)
                        for pi, pts in enumerate(pairs):
                            msg = msgs[pi]
                            for k, t in enumerate(pts):
                                tl = 2 * pi + k
                                nb = _tile_nodes(t)
                                for ch in range(2):
                                    dc0 = t * D_W + ch * TN
                                    nc.tensor.matmul(
                                        psz[:, tl * TN : tl * TN + nb],
                                        lhsT=msg[:, (2 * k + ch) * 128 : (2 * k + ch + 1) * 128],
                                        rhs=d_sb[:, dc0 : dc0 + nb],
                                        start=(ch == 0),
                                        stop=(ch == 1),
                                    )
                        zin = zp.tile([128, 480], bf16, tag="zin")
                        nc.vector.scalar_tensor_tensor(
                            zin[:, :gw], hT[:, gsl], e1bc[:, l : l + 1], psz[:, :gw],
                            op0=ALU.mult, op1=ALU.add,
                        )
                        psy = pmlp.tile([128, 480], f32, tag="y")
                        nc.tensor.matmul(
                            psy[:, :gw], lhsT=w1_l, rhs=zin[:, :gw], start=True, stop=True
                        )
                        y1 = zp.tile([128, 480], bf16, tag="y1")
                        if Y1_PAT[gidx % len(Y1_PAT)] == "A":
                            nc.scalar.activation(y1[:, :gw], psy[:, :gw], AF.Relu, bias=b1_l)
                        else:
                            nc.vector.tensor_scalar(
                                y1[:, :gw], psy[:, :gw], b1_l, 0.0, op0=ALU.add, op1=ALU.max
                            )
                        psz2 = pmlp.tile([128, 480], f32, tag="y")
                        nc.tensor.matmul(
                            psz2[:, :gw], lhsT=w2_l, rhs=y1[:, :gw], start=True, stop=True
                        )
                        if OUT_PAT[gidx % len(OUT_PAT)] == "A":
                            nc.scalar.activation(hT[:, gsl], psz2[:, :gw], AF.Identity, bias=b2_l)
                        else:
                            nc.vector.tensor_scalar(
                                hT[:, gsl], psz2[:, :gw], b2_l, None, op0=ALU.add
                            )
                        # hT -> h_nm transposes (PE + one copy per group)
                        pn = max(_tile_nodes(t) for t in ts)
                        ptr = ptp.tile([128, len(ts) * 128], bf16, tag="tr")
                        for k, t in enumerate(ts):
                            nb, tb = _tile_nodes(t), _tile_base(t)
                            nc.tensor.transpose(
                                ptr[0:nb, k * 128 : (k + 1) * 128],
                                hT[:, tb : tb + nb],
                                id_sb,
                            )
                        ceng = COPY_PAT[gidx % len(COPY_PAT)]
                        dst = h_nm[0:pn, ts[0] * 128 : ts[0] * 128 + len(ts) * 128]
                        if ceng == "A":
                            nc.scalar.activation(dst, ptr[0:pn], AF.Copy)
                        else:
                            nc.vector.tensor_copy(dst, ptr[0:pn])

            # ---------------- pooling ----------------
            with (
                tc.tile_pool(name="po_sb", bufs=1) as po,
                tc.tile_pool(name="ps_hs", bufs=1, space="PSUM") as phs,
                tc.tile_pool(name="ps_sm1", bufs=1, space="PSUM") as psm_q,
                tc.tile_pool(name="ps_o", bufs=1, space="PSUM") as pso,
            ):
                hs = phs.tile([128, S_LOC], f32, tag="hs")
                for t in range(NT):
                    s0 = t * SG_T
                    sw = min(SG_T, S_LOC - s0)
                    nc.tensor.matmul(
                        hs[:, s0 : s0 + sw],
                        lhsT=h_nm[:, t * 128 : (t + 1) * 128],
                        rhs=p1_sb[:, s0 : s0 + sw],
                        start=True,
                        stop=True,
                    )
                wt = w_bc
                nc.vector.tensor_tensor(wt, hs, w_bc, ALU.mult)
                nc.vector.tensor_reduce(
                    ndT,
                    wt.rearrange("p (a b) -> p a b", b=M_SUB),
                    AX.X,
                    ALU.add,
                )
                ndTb = po.tile([128, NCAN_LOC], bf16, tag="ndTb")
                nc.vector.tensor_tensor(ndTb, ndT, rbc, ALU.mult)
                pout = pso.tile([NUM_GRAPHS, H], f32, tag="po")
                for q in range(NQ):
                    ptq = psm_q.tile([128, 128], bf16, tag="pq")
                    nc.tensor.transpose(ptq, ndTb[:, q * 128 : (q + 1) * 128], id_sb)
                    nnm = po.tile([128, 128], bf16, tag="nnm")
                    nc.vector.tensor_copy(nnm, ptq)
                    nc.tensor.matmul(
                        pout,
                        lhsT=g_sb[:, q * NUM_GRAPHS : (q + 1) * NUM_GRAPHS],
                        rhs=nnm,
                        start=(q == 0),
                        stop=(q == NQ - 1),
                    )
                outs = po.tile([NUM_GRAPHS, H], f32, tag="outs")
                nc.scalar.activation(outs, pout, AF.Copy)
                nc.sync.dma_start(out=out_d, in_=outs)

    with TileContext(nc) as tc:
        if repeat > 1:
            with tc.For_i(0, repeat, 1) as _i:
                _kernel_body(tc)
        else:
            _kernel_body(tc)

    nc.finalize()
    return nc


_CACHE = {}


def _get_bass():
    if "nc" not in _CACHE:
        _CACHE["nc"] = _build_bass()
    return _CACHE["nc"]


def kernel(**inputs):
    from concourse.bass_utils import run_bass_kernel_spmd

    per_core, shared = _host_preprocess(inputs)
    in_maps = [{**pc, **shared} for pc in per_core]
    nc = _get_bass()
    res = run_bass_kernel_spmd(nc, in_maps, core_ids=list(range(NCORES)))
    out = np.zeros((NUM_GRAPHS, H), dtype=np.float32)
    for r in res.results:
        out += np.asarray(r["out"], dtype=np.float32)
    return out


# revision 5
# speedup vs baseline: 14.9449x; 4.9230x over previous
"""Trainium2 Bass kernel for nn_Arch7V3GraphEncoder (gnn_message_passing), v7.

v2 + merged gather/bond matmuls: tiles hold 120 nodes (15 subgraphs), and
partitions 120-127 of every h_nm block carry the bond embedding table, so a
single one-hot stationary [128, slots] per chunk computes
h[src]*valid + bond_emb[tok] in one matmul (half the PE instructions of the
separate gather+bond pair). Subgraph pooling columns are globally ordered, so
the 15-subgraph tiling leaves the HT-softmax reduce untouched.
"""

import sys

sys.path.insert(0, "/opt/trn_rl_repo")

import numpy as np
import ml_dtypes

BF16 = ml_dtypes.bfloat16
F8 = ml_dtypes.float8_e4m3

# Problem constants (hardcoded per spec).
N_TOTAL = 4096
M_SUB = 4
K_NODES = 8
L_LAYERS = 4
H = 128
NUM_GRAPHS = 32
IN_CH = 119
EDGE_DIM = 8
S_ALL = N_TOTAL * M_SUB          # 16384 subgraphs
SK_ALL = S_ALL * K_NODES         # 131072 flat nodes
E_ALL = 12 * S_ALL               # 196608 edges
NCORES = 8
S_LOC = S_ALL // NCORES          # 2048 subgraphs / core
SK_LOC = SK_ALL // NCORES        # 16384 flat nodes / core
NCAN_LOC = N_TOTAL // NCORES     # 512 canonical nodes / core
NQ = NCAN_LOC // 128             # 4 canonical chunks of 128
E_CAP = 256                      # edge slots per tile (2 chunks of 128)

SG_T = 15                        # subgraphs per tile
TN = SG_T * K_NODES              # 120 nodes per tile
NT = (S_LOC + SG_T - 1) // SG_T  # 137 tiles (last tile has 8 subgraphs)
D_W = 2 * TN                     # d_oh columns per tile (2 chunks x 120)

# node-column base and node count of tile t
def _tile_base(t):
    return t * TN


def _tile_nodes(t):
    return min(TN, SK_LOC - t * TN)


# groups of up to 4 tiles for the MLP/aggregation stage
GROUPS = []
_t = 0
while _t < NT:
    ts = list(range(_t, min(_t + 4, NT)))
    GROUPS.append(ts)
    _t += 4

# ---- tuning knobs ----
RELU_PAT = "A"
Y1_PAT = "A"
OUT_PAT = "D"
COPY_PAT = "D"
S_CHUNKS = 8
X_CHUNK = 512
PSUM_M = 3
PSUM_Z = 2
PSUM_Y = 2
PSUM_T = 1
BULK_ENG = "S"       # S=SP HWDGE, G=gpsimd SWDGE for bulk input loads


def _host_preprocess(inputs):
    x_tokens = np.asarray(inputs["x_tokens"]).astype(np.int64)
    edge_tokens = np.asarray(inputs["edge_tokens"]).astype(np.int64)
    intra_ei = np.asarray(inputs["intra_ei"]).astype(np.int64)
    node_ids = np.asarray(inputs["node_ids"]).astype(np.int64)
    valid = np.asarray(inputs["valid"]).astype(bool)
    log_probs = np.asarray(inputs["log_probs"]).astype(np.float32)
    batch_graph = np.asarray(inputs["batch_graph"]).astype(np.int64)

    src, dst = intra_ei[0], intra_ei[1]
    e_sub = src // K_NODES
    assert np.array_equal(dst // K_NODES, e_sub), "edges must be intra-subgraph"

    core_of_e = e_sub // S_LOC
    sub_loc_e = e_sub % S_LOC
    tile_of_e = sub_loc_e // SG_T
    key = core_of_e * NT + tile_of_e
    counts = np.bincount(key, minlength=NCORES * NT)
    assert counts.max() <= E_CAP, f"edge overflow: {counts.max()} > {E_CAP}"

    order = np.argsort(key, kind="stable")
    starts = np.zeros(NCORES * NT, dtype=np.int64)
    starts[1:] = np.cumsum(counts)[:-1]
    slot = np.empty(E_ALL, dtype=np.int64)
    slot[order] = np.arange(E_ALL) - starts[key[order]]

    ec = NT * E_CAP                       # merged s/bond one-hot columns
    ecd = NT * D_W                        # d one-hot columns
    j_of = lambda s: s                    # clarity
    src_loc = (src % SK_LOC) - tile_of_e * TN
    dst_loc = (dst % SK_LOC) - tile_of_e * TN
    col = tile_of_e * E_CAP + slot
    chunk = slot // 128
    e_loc = slot % 128
    dcol = tile_of_e * D_W + chunk * TN + dst_loc

    # src one-hot is built on device from a column index row; invalid
    # edges get 255 (matches no partition) so the valid mask folds in free
    sidx = np.full((NCORES, 1, ec), 255.0, dtype=BF16)
    sidx[core_of_e, 0, col] = np.where(valid[src], src_loc, 255).astype(BF16)
    bond8 = np.zeros((NCORES, 8, ec), dtype=F8)
    bond8[core_of_e, edge_tokens, col] = np.asarray(1, dtype=F8)
    # the dst one-hot is built on device from a compact index map:
    # didx[slot%128, tile*2+chunk] = dst_loc (255 = empty slot)
    didx = np.full((NCORES, 128, NT * 2), 255.0, dtype=BF16)
    didx[core_of_e, e_loc, tile_of_e * 2 + chunk] = dst_loc.astype(BF16)
    colio8 = np.tile(np.arange(TN, dtype=np.float32), 8).reshape(1, 8 * TN)
    colio8 = colio8.repeat(128, 0).astype(BF16)

    j = np.arange(SK_ALL)
    j_core = j // SK_LOC
    j_loc = j % SK_LOC
    xidx = x_tokens.reshape(NCORES, 1, SK_LOC).astype(BF16)

    # Subgraph pooling one-hot: node row within tile -> global subgraph col
    vm = node_ids >= 0
    tile_of_j = j_loc // TN
    row_of_j = j_loc - tile_of_j * TN
    P1 = np.zeros((NCORES, 128, S_LOC), dtype=BF16)
    P1[j_core, row_of_j, j_loc // K_NODES] = vm.astype(BF16)
    cnt = np.bincount(j // K_NODES, weights=vm.astype(np.float64), minlength=S_ALL)
    recip_cnt = (1.0 / np.maximum(cnt, 1.0)).astype(np.float32).reshape(NCORES, 1, S_LOC)

    n = np.arange(N_TOTAL)
    Gmat = np.zeros((NCORES, 128, NQ * NUM_GRAPHS), dtype=BF16)
    Gmat[n // NCAN_LOC, n % 128, ((n % NCAN_LOC) // 128) * NUM_GRAPHS + batch_graph] = (
        np.asarray(1, dtype=BF16)
    )

    lp = log_probs.reshape(NCORES, 1, S_LOC).astype(np.float32)

    atom_emb = np.asarray(inputs["atom_emb"]).astype(np.float32)
    role_emb = np.asarray(inputs["role_emb"]).astype(np.float32)
    # reference: role = role_emb[is_root] -> roots get row 1, others row 0
    atom2 = np.zeros((128, H), dtype=BF16)
    atom2[:IN_CH] = (atom_emb + role_emb[0]).astype(BF16)
    diff = (role_emb[1] - role_emb[0]).reshape(1, H).astype(BF16)
    rootmask = (np.arange(X_CHUNK) % K_NODES == 0).reshape(1, X_CHUNK).astype(BF16)

    bond = np.asarray(inputs["bond_emb"]).astype(BF16)      # [8, H]
    bond_tiled = np.tile(bond[:, None, :], (1, NT, 1)).reshape(8, NT * H)

    w1 = np.asarray(inputs["mlp_w1"]).astype(BF16)
    w2 = np.asarray(inputs["mlp_w2"]).astype(BF16)
    wpack = np.concatenate(
        [w1.transpose(1, 0, 2).reshape(H, L_LAYERS * H),
         w2.transpose(1, 0, 2).reshape(H, L_LAYERS * H)], axis=1
    )
    bpack = np.concatenate(
        [np.asarray(inputs["mlp_b1"]).astype(np.float32).T,
         np.asarray(inputs["mlp_b2"]).astype(np.float32).T], axis=1
    )

    per_core = []
    for c in range(NCORES):
        per_core.append(
            {
                "sidx": np.ascontiguousarray(sidx[c]),
                "bond8": np.ascontiguousarray(bond8[c]),
                "didx": np.ascontiguousarray(didx[c]),
                "xidx": np.ascontiguousarray(xidx[c]),
                "p1": np.ascontiguousarray(P1[c]),
                "gmat": np.ascontiguousarray(Gmat[c]),
                "recip_cnt": np.ascontiguousarray(recip_cnt[c]),
                "lp": np.ascontiguousarray(lp[c]),
            }
        )

    shared = {
        "colio8": colio8,
        "iota128": np.arange(128, dtype=np.float32).reshape(128, 1),
        "onesb": np.ones((1, 128), dtype=BF16),
        "atom2": atom2,
        "diff": diff,
        "rootmask": rootmask,
        "bond_tiled": np.ascontiguousarray(bond_tiled),
        "wpack": np.ascontiguousarray(wpack),
        "bpack": np.ascontiguousarray(bpack),
        "eps": np.asarray(inputs["eps"]).astype(np.float32).reshape(1, L_LAYERS),
        "alpha": np.asarray(inputs["ht_alpha"]).astype(np.float32).reshape(1, 1),
        "ones128": np.ones((1, 128), dtype=np.float32),
        "ident": np.eye(128, dtype=BF16),
    }
    return per_core, shared


def _build_bass(repeat=1):
    import concourse.bass as bass
    import concourse.mybir as mybir
    from concourse import bacc
    from concourse.tile import TileContext

    f32 = mybir.dt.float32
    bf16 = mybir.dt.bfloat16
    fp8 = mybir.dt.float8e4
    AF = mybir.ActivationFunctionType
    ALU = mybir.AluOpType
    AX = mybir.AxisListType

    ec = NT * E_CAP
    ecd = NT * D_W

    nc = bacc.Bacc("TRN2", target_bir_lowering=False, debug=False, num_devices=NCORES)

    def din(name, shape, dt):
        return nc.dram_tensor(name, shape, dt, kind="ExternalInput").ap()

    si_d = din("sidx", [1, ec], bf16)
    b8_d = din("bond8", [8, ec], fp8)
    di_d = din("didx", [128, NT * 2], bf16)
    cio_d = din("colio8", [128, 8 * TN], bf16)
    x_d = din("xidx", [1, SK_LOC], bf16)
    io_d = din("iota128", [128, 1], f32)
    ob_d = din("onesb", [1, 128], bf16)
    p1_d = din("p1", [128, S_LOC], bf16)
    g_d = din("gmat", [128, NQ * NUM_GRAPHS], bf16)
    rc_d = din("recip_cnt", [1, S_LOC], f32)
    lp_d = din("lp", [1, S_LOC], f32)
    atom_d = din("atom2", [128, H], bf16)
    diff_d = din("diff", [1, H], bf16)
    rm_d = din("rootmask", [1, X_CHUNK], bf16)
    bt_d = din("bond_tiled", [8, NT * H], bf16)
    wp_d = din("wpack", [128, 2 * L_LAYERS * H], bf16)
    bp_d = din("bpack", [128, 2 * L_LAYERS], f32)
    eps_d = din("eps", [1, L_LAYERS], f32)
    al_d = din("alpha", [1, 1], f32)
    ones_d = din("ones128", [1, 128], f32)
    id_d = din("ident", [128, 128], bf16)

    out_d = nc.dram_tensor("out", [NUM_GRAPHS, H], f32, kind="ExternalOutput").ap()

    def _kernel_body(tc):
        bulk = nc.sync if BULK_ENG == "S" else nc.gpsimd
        with tc.tile_pool(name="persist", bufs=1) as pp:
            sb_sb = pp.tile([128, ec], fp8, tag="s")
            d_sb = pp.tile([128, ecd], fp8, tag="d")
            di_sb = pp.tile([128, NT * 2], bf16, tag="di")
            io_sb = pp.tile([128, 1], f32, tag="io")
            ob_sb = pp.tile([1, 128], bf16, tag="ob")
            cio_sb = pp.tile([128, 8 * TN], bf16, tag="cio")
            hT = pp.tile([128, SK_LOC], bf16, tag="hT")
            h_nm = pp.tile([128, NT * 128], bf16, tag="hnm")
            p1_sb = pp.tile([128, S_LOC], bf16, tag="p1")
            g_sb = pp.tile([128, NQ * NUM_GRAPHS], bf16, tag="g")
            atom_sb = pp.tile([128, H], bf16, tag="atom")
            diff_sb = pp.tile([1, H], bf16, tag="diff")
            rm_sb = pp.tile([1, X_CHUNK], bf16, tag="rm")
            wp_sb = pp.tile([128, 2 * L_LAYERS * H], bf16, tag="wp")
            bp_sb = pp.tile([128, 2 * L_LAYERS], f32, tag="bp")
            eps_sb = pp.tile([1, L_LAYERS], f32, tag="eps")
            e1bc = pp.tile([128, L_LAYERS], f32, tag="e1bc")
            al_sb = pp.tile([1, 1], f32, tag="al")
            ones_sb = pp.tile([1, 128], f32, tag="ones")
            id_sb = pp.tile([128, 128], bf16, tag="id")
            w_bc = pp.tile([128, S_LOC], f32, tag="wbc")
            rbc = pp.tile([128, S_LOC // M_SUB], f32, tag="rbc")
            ndT = pp.tile([128, NCAN_LOC], f32, tag="ndT")

            bulk.dma_start(out=wp_sb, in_=wp_d)
            bulk.dma_start(out=bp_sb, in_=bp_d)
            bulk.dma_start(out=eps_sb, in_=eps_d)
            bulk.dma_start(out=ones_sb, in_=ones_d)
            bulk.dma_start(out=id_sb, in_=id_d)
            # bond rows of every h_nm block, written once
            bulk.dma_start(out=h_nm[120:128, :NT * H], in_=bt_d)
            # rows 64..119 of the short last tile are never written by the
            # transposes; zero them so stray NaNs can't leak through the
            # (zero-weighted) gather/pool contractions
            if _tile_nodes(NT - 1) < TN:
                nc.gpsimd.memset(
                    h_nm[_tile_nodes(NT - 1) : TN, (NT - 1) * 128 : NT * 128], 0
                )
            bulk.dma_start(out=di_sb, in_=di_d)
            bulk.dma_start(out=io_sb, in_=io_d)
            bulk.dma_start(out=ob_sb, in_=ob_d)
            bulk.dma_start(out=cio_sb, in_=cio_d)
            # build the dst one-hot on device: one is_equal per 8-chunk swath
            nch_all = NT * 2
            sw0 = 0
            while sw0 < nch_all:
                K = min(8, nch_all - sw0)
                nc.vector.tensor_tensor(
                    d_sb[:, sw0 * TN : (sw0 + K) * TN].rearrange(
                        "p (a b) -> p a b", b=TN
                    ),
                    cio_sb[:, : K * TN].rearrange("p (a b) -> p a b", b=TN),
                    di_sb[:, sw0 : sw0 + K].broadcast_to([128, K, TN]),
                    ALU.is_equal,
                )
                sw0 += K
            bulk.dma_start(out=sb_sb[120:128, :], in_=b8_d)
            bulk.dma_start(out=p1_sb, in_=p1_d)
            bulk.dma_start(out=g_sb, in_=g_d)
            bulk.dma_start(out=al_sb, in_=al_d)
            nc.sync.dma_start(out=atom_sb, in_=atom_d)
            nc.sync.dma_start(out=diff_sb, in_=diff_d)
            nc.sync.dma_start(out=rm_sb, in_=rm_d)

            # ---------------- embed ----------------
            with (
                tc.tile_pool(name="emb_sb", bufs=3) as ep,
                tc.tile_pool(name="sm_sb", bufs=1) as smp,
                tc.tile_pool(name="emb_ps", bufs=2, space="PSUM") as epp,
                tc.tile_pool(name="emb_ps1", bufs=1, space="PSUM") as epp1,
                tc.tile_pool(name="emb_psb", bufs=3, space="PSUM") as eppb,
                tc.tile_pool(name="emb_ptr", bufs=1, space="PSUM") as eptr,
            ):
                s0 = 0
                while s0 < ec:
                    w = min(512, ec - s0)
                    ssl = slice(s0, s0 + w)
                    srow = ep.tile([1, 512], bf16, tag="sr")
                    nc.sync.dma_start(out=srow[:, :w], in_=si_d[:, ssl])
                    psb = eppb.tile([128, 512], f32, tag="xb")
                    nc.tensor.matmul(
                        psb[:, :w], lhsT=ob_sb, rhs=srow[:, :w], start=True, stop=True
                    )
                    nc.vector.tensor_scalar(
                        sb_sb[0:120, ssl], psb[0:120, :w], io_sb[0:120, 0:1],
                        None, op0=ALU.is_equal,
                    )
                    s0 += w
                pse = epp1.tile([128, L_LAYERS], f32, tag="pse")
                nc.tensor.matmul(pse, lhsT=ones_sb, rhs=eps_sb, start=True, stop=True)
                nc.scalar.activation(e1bc, pse, AF.Copy, bias=1.0)
                rc_sb = smp.tile([1, S_LOC], f32, tag="rc")
                lp_sb = smp.tile([1, S_LOC], f32, tag="lp")
                bulk.dma_start(out=rc_sb, in_=rc_d)
                bulk.dma_start(out=lp_sb, in_=lp_d)
                nc.vector.tensor_scalar(
                    lp_sb, lp_sb, al_sb[:, 0:1], -1.0, op0=ALU.mult, op1=ALU.mult
                )
                nc.scalar.activation(lp_sb, lp_sb, AF.Exp)
                et = lp_sb
                s4 = smp.tile([1, S_LOC // M_SUB], f32, tag="s4")
                nc.vector.tensor_reduce(
                    s4, et.rearrange("p (a b) -> p a b", b=M_SUB), AX.X, ALU.add
                )
                r4 = smp.tile([1, S_LOC // M_SUB], f32, tag="r4")
                nc.vector.reciprocal(r4, s4)
                nc.vector.tensor_tensor(et, et, rc_sb, ALU.mult)
                wr = et
                for q in range(S_LOC // 512):
                    pw = epp1.tile([128, 512], f32, tag="pw")
                    nc.tensor.matmul(
                        pw, lhsT=ones_sb, rhs=wr[:, q * 512 : (q + 1) * 512],
                        start=True, stop=True,
                    )
                    nc.vector.tensor_copy(w_bc[:, q * 512 : (q + 1) * 512], pw)
                pw = epp1.tile([128, 512], f32, tag="pw")
                nc.tensor.matmul(pw, lhsT=ones_sb, rhs=r4, start=True, stop=True)
                nc.vector.tensor_copy(rbc, pw[:, : S_LOC // M_SUB])

                for q in range(SK_LOC // X_CHUNK):
                    qsl = slice(q * X_CHUNK, (q + 1) * X_CHUNK)
                    xt = ep.tile([128, X_CHUNK], fp8, tag="x")
                    xrow = ep.tile([1, X_CHUNK], bf16, tag="xr")
                    nc.sync.dma_start(out=xrow, in_=x_d[:, qsl])
                    pxb = eppb.tile([128, X_CHUNK], f32, tag="xb")
                    nc.tensor.matmul(
                        pxb, lhsT=ob_sb, rhs=xrow, start=True, stop=True
                    )
                    nc.vector.tensor_scalar(
                        xt, pxb, io_sb[:, 0:1], None, op0=ALU.is_equal
                    )
                    ps = epp.tile([128, X_CHUNK], f32, tag="ps")
                    nc.tensor.matmul(ps, lhsT=atom_sb, rhs=xt, start=True, stop=False)
                    nc.tensor.matmul(ps, lhsT=diff_sb, rhs=rm_sb, start=False, stop=True)
                    nc.scalar.activation(hT[:, qsl], ps, AF.Copy)
                # hT -> h_nm blocks (120-node tiles) per 4-tile group
                for gi, ts in enumerate(GROUPS):
                    pn = max(_tile_nodes(t) for t in ts)
                    ptr = eptr.tile([128, len(ts) * 128], bf16, tag="ptr")
                    for k, t in enumerate(ts):
                        nb, n0 = _tile_nodes(t), _tile_base(t)
                        nc.tensor.transpose(
                            ptr[0:nb, k * 128 : (k + 1) * 128],
                            hT[:, n0 : n0 + nb],
                            id_sb,
                        )
                    nc.vector.tensor_copy(
                        h_nm[0:pn, ts[0] * 128 : ts[0] * 128 + len(ts) * 128],
                        ptr[0:pn],
                    )

            # ---------------- layers ----------------
            with (
                tc.tile_pool(name="msg_sb", bufs=3) as mp,
                tc.tile_pool(name="zy_sb", bufs=3) as zp,
                tc.tile_pool(name="ps_m", bufs=PSUM_M, space="PSUM") as pm,
                tc.tile_pool(name="ps_z", bufs=PSUM_Z, space="PSUM") as pz,
                tc.tile_pool(name="ps_mlp", bufs=PSUM_Y, space="PSUM") as pmlp,
                tc.tile_pool(name="ps_tr", bufs=PSUM_T, space="PSUM") as ptp,
            ):
                for l in range(L_LAYERS):
                    w1_l = wp_sb[:, l * H : (l + 1) * H]
                    w2_l = wp_sb[:, (L_LAYERS + l) * H : (L_LAYERS + l + 1) * H]
                    b1_l = bp_sb[:, l : l + 1]
                    b2_l = bp_sb[:, L_LAYERS + l : L_LAYERS + l + 1]
                    for gidx, ts in enumerate(GROUPS):
                        n0 = _tile_base(ts[0])
                        gw = sum(_tile_nodes(t) for t in ts)
                        gsl = slice(n0, n0 + gw)
                        psz = pz.tile([128, 480], f32, tag="z")
                        # pairs of tiles: merged gather+bond, then relu, then
                        # scatter (gathers of both pairs emitted first)
                        pairs = [ts[i : i + 2] for i in range(0, len(ts), 2)]
                        msgs = []
                        for pi, pts in enumerate(pairs):
                            psm = pm.tile([128, 512], f32, tag="m")
                            for k, t in enumerate(pts):
                                for ch in range(2):
                                    c0 = t * E_CAP + ch * 128
                                    osl = slice((2 * k + ch) * 128, (2 * k + ch + 1) * 128)
                                    nc.tensor.matmul(
                                        psm[:, osl],
                                        lhsT=sb_sb[:, c0 : c0 + 128],
                                        rhs=h_nm[:, t * 128 : (t + 1) * 128],
                                        start=True,
                                        stop=True,
                                    )
                            msg = mp.tile([128, 512], bf16, tag="msg")
                            msgs.append(msg)
                            eng = RELU_PAT[(gidx * 2 + pi) % len(RELU_PAT)]
                            w = len(pts) * 256
                            if eng == "A":
                                nc.scalar.activation(msg[:, :w], psm[:, :w], AF.Relu)
                            else:
                                nc.vector.tensor_scalar_max(msg[:, :w], psm[:, :w], 0.0)
                        for pi, pts in enumerate(pairs):
                            msg = msgs[pi]
                            for k, t in enumerate(pts):
                                tl = 2 * pi + k
                                nb = _tile_nodes(t)
                                for ch in range(2):
                                    dc0 = t * D_W + ch * TN
                                    nc.tensor.matmul(
                                        psz[:, tl * TN : tl * TN + nb],
                                        lhsT=msg[:, (2 * k + ch) * 128 : (2 * k + ch + 1) * 128],
                                        rhs=d_sb[:, dc0 : dc0 + nb],
                                        start=(ch == 0),
                                        stop=(ch == 1),
                                    )
                        zin = zp.tile([128, 480], bf16, tag="zin")
                        nc.vector.scalar_tensor_tensor(
                            zin[:, :gw], hT[:, gsl], e1bc[:, l : l + 1], psz[:, :gw],
                            op0=ALU.mult, op1=ALU.add,
                        )
                        psy = pmlp.tile([128, 480], f32, tag="y")
                        nc.tensor.matmul(
                            psy[:, :gw], lhsT=w1_l, rhs=zin[:, :gw], start=True, stop=True
                        )
                        y1 = zp.tile([128, 480], bf16, tag="y1")
                        if Y1_PAT[gidx % len(Y1_PAT)] == "A":
                            nc.scalar.activation(y1[:, :gw], psy[:, :gw], AF.Relu, bias=b1_l)
                        else:
                            nc.vector.tensor_scalar(
                                y1[:, :gw], psy[:, :gw], b1_l, 0.0, op0=ALU.add, op1=ALU.max
                            )
                        psz2 = pmlp.tile([128, 480], f32, tag="y")
                        nc.tensor.matmul(
                            psz2[:, :gw], lhsT=w2_l, rhs=y1[:, :gw], start=True, stop=True
                        )
                        if OUT_PAT[gidx % len(OUT_PAT)] == "A":
                            nc.scalar.activation(hT[:, gsl], psz2[:, :gw], AF.Identity, bias=b2_l)
                        else:
                            nc.vector.tensor_scalar(
                                hT[:, gsl], psz2[:, :gw], b2_l, None, op0=ALU.add
                            )
                        # hT -> h_nm transposes (PE + one copy per group)
                        pn = max(_tile_nodes(t) for t in ts)
                        ptr = ptp.tile([128, len(ts) * 128], bf16, tag="tr")
                        for k, t in enumerate(ts):
                            nb, tb = _tile_nodes(t), _tile_base(t)
                            nc.tensor.transpose(
                                ptr[0:nb, k * 128 : (k + 1) * 128],
                                hT[:, tb : tb + nb],
                                id_sb,
                            )
                        ceng = COPY_PAT[gidx % len(COPY_PAT)]
                        dst = h_nm[0:pn, ts[0] * 128 : ts[0] * 128 + len(ts) * 128]
                        if ceng == "A":
                            nc.scalar.activation(dst, ptr[0:pn], AF.Copy)
                        else:
                            nc.vector.tensor_copy(dst, ptr[0:pn])

            # ---------------- pooling ----------------
            with (
                tc.tile_pool(name="po_sb", bufs=1) as po,
                tc.tile_pool(name="ps_hs", bufs=1, space="PSUM") as phs,
                tc.tile_pool(name="ps_sm1", bufs=1, space="PSUM") as psm_q,
                tc.tile_pool(name="ps_o", bufs=1, space="PSUM") as pso,
            ):
                hs = phs.tile([128, S_LOC], f32, tag="hs")
                for t in range(NT):
                    s0 = t * SG_T
                    sw = min(SG_T, S_LOC - s0)
                    nc.tensor.matmul(
                        hs[:, s0 : s0 + sw],
                        lhsT=h_nm[:, t * 128 : (t + 1) * 128],
                        rhs=p1_sb[:, s0 : s0 + sw],
                        start=True,
                        stop=True,
                    )
                wt = w_bc
                nc.vector.tensor_tensor(wt, hs, w_bc, ALU.mult)
                nc.vector.tensor_reduce(
                    ndT,
                    wt.rearrange("p (a b) -> p a b", b=M_SUB),
                    AX.X,
                    ALU.add,
                )
                ndTb = po.tile([128, NCAN_LOC], bf16, tag="ndTb")
                nc.vector.tensor_tensor(ndTb, ndT, rbc, ALU.mult)
                pout = pso.tile([NUM_GRAPHS, H], f32, tag="po")
                for q in range(NQ):
                    ptq = psm_q.tile([128, 128], bf16, tag="pq")
                    nc.tensor.transpose(ptq, ndTb[:, q * 128 : (q + 1) * 128], id_sb)
                    nnm = po.tile([128, 128], bf16, tag="nnm")
                    nc.vector.tensor_copy(nnm, ptq)
                    nc.tensor.matmul(
                        pout,
                        lhsT=g_sb[:, q * NUM_GRAPHS : (q + 1) * NUM_GRAPHS],
                        rhs=nnm,
                        start=(q == 0),
                        stop=(q == NQ - 1),
                    )
                outs = po.tile([NUM_GRAPHS, H], f32, tag="outs")
                nc.scalar.activation(outs, pout, AF.Copy)
                nc.sync.dma_start(out=out_d, in_=outs)

    with TileContext(nc) as tc:
        if repeat > 1:
            with tc.For_i(0, repeat, 1) as _i:
                _kernel_body(tc)
        else:
            _kernel_body(tc)

    nc.finalize()
    return nc


_CACHE = {}


def _get_bass():
    if "nc" not in _CACHE:
        _CACHE["nc"] = _build_bass()
    return _CACHE["nc"]


def kernel(**inputs):
    from concourse.bass_utils import run_bass_kernel_spmd

    per_core, shared = _host_preprocess(inputs)
    in_maps = [{**pc, **shared} for pc in per_core]
    nc = _get_bass()
    res = run_bass_kernel_spmd(nc, in_maps, core_ids=list(range(NCORES)))
    out = np.zeros((NUM_GRAPHS, H), dtype=np.float32)
    for r in res.results:
        out += np.asarray(r["out"], dtype=np.float32)
    return out
